# revision 31
# baseline (speedup 1.0000x reference)
"""Trainium2 Bass kernel for nn_Attention_39934605918652.

res[b] = W0 @ x0[b] + sum_{n=1..N-1} W2 @ tanh(W1a @ x0[b] + W1b @ x[b,n])

Algebraic optimization: W2 is n-independent, so
    sum_n W2 @ tanh(...) = W2 @ (sum_n tanh(...))
leaving one [B,H]x[H,F] epilogue matmul.

Sharding: data-parallel over batch B=128 across 8 cores (16 batches/core),
weights replicated. No collectives.

v2 design (vs the 88us bf16 baseline): the three dominant costs were the
PE matmul stream (~55us bf16), the ACT tanh stream (129 per-batch calls,
~60us with per-call bubbles), and the DVE segmented reduce (~39us).
 - Main matmuls run in fp8e4m3 with MatmulPerfMode.DoubleRow: operands
   packed [128, ksub, cols] with contraction f = ksub*128 + p; each
   instruction contracts 2 ksubs (256 features).  W1b is pre-scaled by 32
   on host so its ~N(0, 1/1024) entries use e4m3's normal range; the tanh
   activation applies scale=1/32 to compensate.
 - The h0 bias is injected INTO PSUM with rank-1 fp16 matmuls
   (lhsT = h0row [1,128] at partition 0, rhs = ones [1,256]), so no
   separate elementwise bias pass exists and ACT needs no per-batch bias.
   h0 is computed batch-major ([16,1024] psum), cast/scaled to fp16 by one
   ACT copy, and flattened to partition 0 by one SBUF->SBUF DMA (engines
   can only address SBUF partition starts 0/32/64/96, so a [1, B*H] view
   is the only legal per-batch lhsT source).
 - tanh is ONE big ACT call per (h,q) psum tile [128,1024] (pad column
   included; excluded from the reduce), PSUM -> bf16 SBUF.
 - The segmented reduce_sum alternates between the DVE and the otherwise
   idle Pool (gpsimd) engine.
Measured end-to-end rel err vs a float64 oracle: ~2.5e-3 (fp8 per-element
noise averages down over the 255-term n-sum; harness gate is 2e-2).

Device layout (per core):
  xiT   [128, 4q*(4k*1024c)] fp8  col c = 256*b_in_q + n, f = k*128 + p
  x0T   [128, 4*BL]          fp16 host-packed f-chunks side by side
  w1bT  [128, 4k*1024h]      fp8  = 32*W1b.T packed like xiT
  w1aT  [512, 1024]          fp16 = W1a.T (rhs of batch-major h0 matmul)
  w2T   [1024, 512]          fp16 = W2.T
  w0T   [512, 512]           fp16 = W0.T
Output res [BL=16, 512] f32 per core; host concatenates.
"""

import os
import numpy as np
from contextlib import ExitStack

import concourse.bass as bass
import concourse.tile as tile
from concourse import bacc, mybir
from concourse.bass_utils import run_bass_kernel_spmd

N_CORES = 8
B, N, F, H = 128, 256, 512, 1024
BL = B // N_CORES          # 16 batches per core
NI = N - 1                 # 255 real columns per batch
NP = 256                   # padded columns per batch
NF = F // 128              # 4 f-chunks (= ksubs)
NH = H // 128              # 8 h-tiles
QUADS = BL // 4            # 4 batch-quads; per quad psum tile [128, 4*256]
W1B_SCALE = 32.0           # host pre-scale on W1b before fp8 quantization

F32 = mybir.dt.float32
BF16 = mybir.dt.bfloat16
F16 = mybir.dt.float16
F8 = mybir.dt.float8e4
DR = mybir.MatmulPerfMode.DoubleRow
DRSWI = mybir.MatmulPerfMode.DoubleRowSwInterleave

# How many of the first psum tiles get their bias matmuls issued up front:
# they depend only on h0 (ready ~3us in), so they keep the PE busy/warm
# while xi still streams from HBM. Max useful = psum bufs.
N_PREBIAS = int(os.environ.get("KB_PREBIAS", "2"))
# Dummy fp32 matmuls on zeros (no DMA dependency): HAM only grants the PE
# full clock after ~6us of sustained activity and re-throttles to half
# rate after a multi-us idle gap, so the lead-in gaps are bridged with
# busywork. N_WARM runs before phase 1 (PE idle 6->11us while w1a
# streams); N_WARM2 runs after the prebias block (PE idle ~14->17us
# while xi0 streams). Each is ~427ns warm / ~850ns at half rate.
N_WARM = int(os.environ.get("KB_WARM", "12"))
N_WARM2 = int(os.environ.get("KB_WARM2", "3"))


def _build_kernel():
    nc = bacc.Bacc(
        "TRN2", target_bir_lowering=False, debug=False, num_devices=N_CORES
    )

    xiT = nc.dram_tensor("xiT", [128, QUADS * NF * 1024], F8, kind="ExternalInput").ap()
    w1bT = nc.dram_tensor("w1bT", [128, NF * H], F8, kind="ExternalInput").ap()
    waT = nc.dram_tensor("waT", [128, 64 + NF * 1024], F16, kind="ExternalInput").ap()
    wbT = nc.dram_tensor("wbT", [128, 12 * 512], F16, kind="ExternalInput").ap()
    selT = nc.dram_tensor("selT", [BL, 8 * 512], F16, kind="ExternalInput").ap()
    res = nc.dram_tensor("res", [BL, F], F32, kind="ExternalOutput").ap()

    with tile.TileContext(nc) as tc:
        with ExitStack() as ctx:
            _kernel_body(ctx, tc, xiT, w1bT, waT, wbT, selT, res)

    nc.compile()
    return nc


def _kernel_body(ctx, tc, xiT, w1bT, waT, wbT, selT, res):
    nc = tc.nc
    Tanh = mybir.ActivationFunctionType.Tanh
    Copy = mybir.ActivationFunctionType.Copy

    wpool = ctx.enter_context(tc.tile_pool(name="weights", bufs=1))

    # DMA issue order = first-need order. x0+w1a ride one packed tensor
    # (wa) and w2+w0 another (wb): one descriptor-gen each and 8-12KB
    # per-partition runs instead of many small strided transfers.
    sel_sb = wpool.tile([BL, 8 * 512], F16, tag="sel", name="sel")
    nc.sync.dma_start(sel_sb[:], selT[:])
    # A single descriptor set runs latency-bound (~170GB/s: 8 descriptors
    # per engine, serial); concurrent sets pipeline to ~380GB/s. Critical
    # tensors are therefore split into two concurrent half-transfers.
    wa = wpool.tile([128, 64 + NF * 1024], F16, tag="wa", name="wa")
    WAH = (64 + NF * 1024) // 2
    nc.sync.dma_start(wa[:, :WAH], waT[:, :WAH])
    nc.sync.dma_start(wa[:, WAH:], waT[:, WAH:])
    x0_sb = [wa[:, f * BL : (f + 1) * BL] for f in range(NF)]  # cols 0:64
    w1a_sl = lambda f, hh: wa[:, 64 + f * 1024 + hh * 512 : 64 + f * 1024 + (hh + 1) * 512]
    # Tiny gate reads stall the sync queue until an earlier tensor
    # completes, so later descriptor sets don't steal bandwidth from
    # first-needed tensors.
    gate16 = wpool.tile([1, 64], F16, tag="gate16", name="gate16")
    gate8 = wpool.tile([1, 64], F8, tag="gate8", name="gate8")
    _gate_n = [0]

    def gate_on(tile_ap):
        g = _gate_n[0]
        _gate_n[0] += 1
        dst = gate8 if tile_ap.dtype == F8 else gate16
        nc.sync.dma_start(dst[0:1, g * 4 : g * 4 + 2], tile_ap)
    w1b_all = wpool.tile([128, NF * H], F8, tag="w1b", name="w1b_all")
    nc.sync.dma_start(w1b_all[:], w1bT[:])
    w1b_v = w1b_all[:].rearrange("p (k h) -> p k h", k=NF)
    gate_on(wa[0:1, 0:2])
    gate_on(wa[0:1, WAH : WAH + 2])
    xi_sb = []
    xi_t = []
    for q in range(QUADS):
        t = wpool.tile([128, NF * 1024], F8, tag=f"xi_{q}", name=f"xi_{q}")
        xi_t.append(t)
        base = q * NF * 1024
        nc.sync.dma_start(t[:], xiT[:, base : base + NF * 1024])
        xi_sb.append(t[:].rearrange("p (k c) -> p k c", k=NF))
        if q == 1:
            gate_on(xi_t[0][0:1, 0:2])
    gate_on(xi_t[2][0:1, 0:2])
    wb = wpool.tile([128, 12 * 512], F16, tag="wb", name="wb")
    nc.sync.dma_start(wb[:], wbT[:])
    w2_sb = [wb[:, h * 512 : (h + 1) * 512] for h in range(NH)]
    w0_sb = [wb[:, (NH + f) * 512 : (NH + f + 1) * 512] for f in range(NF)]

    S_sb = [
        wpool.tile([128, BL], F16, tag=f"S_{h}", name=f"S_{h}")
        for h in range(NH)
    ]
    h0T_sb = wpool.tile([BL, H], F16, tag="h0T", name="h0T")

    # PSUM: main pool 3 x [128,1024]f32 (2 banks each) for the wave tiles
    # and ph0; small pool 2 x 1 bank for warm-up + the epilogue
    # accumulator. 3*2 + 2 = 8 banks.
    ppool = ctx.enter_context(tc.tile_pool(name="ps", bufs=3, space="PSUM"))
    spool = ctx.enter_context(tc.tile_pool(name="pss", bufs=2, space="PSUM"))
    itpool = ctx.enter_context(tc.tile_pool(name="it", bufs=6))

    # ---- Phase 0: preload the tanh ACT table during the DMA lead-in
    # (first ACTIVATE otherwise pays the ~1.3us table load mid-kernel).
    tiny = wpool.tile([128, 1], F32, tag="tiny", name="tiny")
    nc.gpsimd.memset(tiny[:], 0.0)
    nc.scalar.activation(tiny[:], tiny[:], Tanh)

    # ---- Phase 0b: PE warm-up while wa streams (see N_WARM).
    wz = wpool.tile([128, 256], F32, tag="warmz", name="warmz")
    nc.gpsimd.memset(wz[:], 0.0)
    pw = spool.tile([128, 256], F32, tag="pss", name="pwarm")
    for _ in range(N_WARM):
        nc.tensor.matmul(pw[:], wz[:, :128], wz[:], start=True, stop=True)

    # ---- Phase 1 (batch-major): h0T[b,h] = sum_f x0T[f,b] W1aT[f,h];
    # cast to fp16 by an ACT copy (32x psum scale baked in) and flattened
    # to partition 0 by a scalar-queue SBUF->SBUF DMA, so per-(b,htile)
    # rows become legal matmul lhsT slices (SBUF access patterns may only
    # start at partitions 0/32/64/96).
    ph0 = ppool.tile([BL, H], F32, tag="ps", name="ph0")
    for hh in range(2):
        sl = slice(hh * 512, (hh + 1) * 512)
        for f in range(NF):
            nc.tensor.matmul(
                ph0[:, sl],
                x0_sb[f],
                w1a_sl(f, hh),
                start=(f == 0),
                stop=(f == NF - 1),
            )
    nc.scalar.activation(h0T_sb[:], ph0[:], Copy, scale=W1B_SCALE)


    def bias_mms(pb, h, q):
        # One selector matmul per PSUM bank: lhsT = h0T[16 batches, 128 h]
        # (phase-1's natural layout), rhs = a host 0/1 matrix routing batch
        # 4q+2bk to cols [0:255) and 4q+2bk+1 to [256:511) of the bank
        # (pad cols all-zero, so tanh(0)=0 and the reduce needs no mask).
        # Writes the full bank with start=True, zeroing it for the DR
        # accumulation. 2 instructions/tile, no flatten DMA, K=16.
        for bk in range(2):
            m = 2 * q + bk
            nc.tensor.matmul(
                pb[:, bk * 512 : (bk + 1) * 512],
                h0T_sb[:, h * 128 : (h + 1) * 128],
                sel_sb[:, m * 512 : (m + 1) * 512],
                start=True,
                stop=False,
                skip_group_check=True,
            )

    def main_mms(pb, h, q):
        # fp8 DoubleRow: 2 ksub-pairs x 2 col-halves (a matmul output may
        # not cross a PSUM bank), each contracting 256 features over 512
        # psum columns. kp outer so consecutive matmuls share lhsT.
        for kp in range(2):
            for bk in range(2):
                cols = slice(bk * 512, (bk + 1) * 512)
                nc.tensor.matmul(
                    pb[:, cols],
                    w1b_v[:, 2 * kp : 2 * kp + 2, h * 128 : (h + 1) * 128],
                    xi_sb[q][:, 2 * kp : 2 * kp + 2, cols],
                    start=False,
                    stop=(kp == 1),
                    perf_mode=DR,
                    skip_group_check=True,
                )

    def consume(h, q, pb, idx):
        # ACT: one big tanh over the whole psum tile (pad cols are exact
        # zeros). A halving add folds cols [128:256) onto [0:128), then a
        # half-size DVE segmented reduce. Pool (gpsimd) measures
        # ~2.25ns/elem on TENSOR_TENSOR vs DVE 2x_1p at ~0.5ns/elem, so
        # adds alternate between them; free-axis reduce is DVE-only.
        it = itpool.tile([128, 4 * NP], BF16, tag="it", name=f"it_{h}_{q}")
        nc.scalar.activation(it[:], pb[:], Tanh, scale=1.0 / W1B_SCALE)
        v = it[:].rearrange("p (b n) -> p b n", b=4)
        half = itpool.tile([128, 4 * 128], BF16, tag="half", name=f"hf_{h}_{q}")
        hv = half[:].rearrange("p (b n) -> p b n", b=4)
        add_eng = nc.gpsimd if (idx % 2 == 0 and idx < 28) else nc.vector
        with nc.allow_low_precision(
            reason="S accumulated in 16-bit to feed the fp16 epilogue matmul"
        ):
            add_eng.tensor_add(hv, v[:, :, 0:128], v[:, :, 128:256])
            nc.vector.reduce_sum(
                S_sb[h][:, q * 4 : (q + 1) * 4],
                hv,
                axis=mybir.AxisListType.X,
            )

    # ---- Phase 2. Tile order: h-pair-major, wave-inner, so each h's four
    # quads finish early and its W2 epilogue matmul can issue mid-stream.
    # The first N_PREBIAS tiles' bias matmuls go up front (they only need
    # h0), and N_WARM2 dummies bridge the PE gap until xi0 lands.
    order = []
    for hp in range(0, NH, 2):
        for w in range(2):
            for h in (hp, hp + 1):
                for q in (2 * w, 2 * w + 1):
                    order.append((h, q))
    done_count = {h: 0 for h in range(NH)}
    w2_pending = []
    po_issued = [0]
    po = spool.tile([BL, F], F32, tag="pss", name="po")

    def po_mm(lhsT, rhs):
        # One shared 12-matmul accumulation group: 8 W2 terms issued as
        # each S[h] completes mid-stream, 4 W0 terms slotted in at idx 17.
        nc.tensor.matmul(
            po[:], lhsT, rhs,
            start=(po_issued[0] == 0),
            stop=(po_issued[0] == NH + NF - 1),
            skip_group_check=True,
        )
        po_issued[0] += 1

    def flush_w2():
        h = w2_pending.pop(0)
        po_mm(S_sb[h][:], w2_sb[h])

    # Warm-up bridge BEFORE the prebias selector matmuls: they wait on the
    # h0T copy (~1.2us of ACT latency after phase 1), and a PE idle gap
    # there can make HAM revoke the full-clock grant for tens of us.
    for _ in range(N_WARM2):
        nc.tensor.matmul(pw[:], wz[:, :128], wz[:], start=True, stop=True)
    pbs = {}
    for h, q in order[:N_PREBIAS]:
        pb = pbs[(h, q)] = ppool.tile([128, 4 * NP], F32, tag="ps", name=f"pb_{h}_{q}")
        bias_mms(pb, h, q)

    for idx, (h, q) in enumerate(order):
        if w2_pending and idx >= 2:
            flush_w2()
        if idx == 17:
            # W0 epilogue (independent of S): x0/w0 are long since loaded;
            # issuing mid-stream keeps it off the tail.
            for f in range(NF):
                po_mm(x0_sb[f], w0_sb[f])
        if (h, q) in pbs:
            pb = pbs.pop((h, q))
        else:
            pb = ppool.tile([128, 4 * NP], F32, tag="ps", name=f"pb_{h}_{q}")
            bias_mms(pb, h, q)
        main_mms(pb, h, q)
        consume(h, q, pb, idx)
        done_count[h] += 1
        if done_count[h] == 4:
            w2_pending.append(h)
    while w2_pending:
        flush_w2()

    # ---- Phase 3 tail: one copy from PSUM (DMA cannot read PSUM), out.
    rt = itpool.tile([BL, F], F32, tag="rt", name="rt")
    nc.vector.tensor_copy(rt[:], po[:])
    nc.sync.dma_start(res[:], rt[:])


_NC_CACHE = {}


def _get_nc():
    key = ("v15", N_PREBIAS, N_WARM, N_WARM2)
    if key not in _NC_CACHE:
        _NC_CACHE[key] = _build_kernel()
    return _NC_CACHE[key]


def _make_in_maps(x, W1, W2, W0):
    import ml_dtypes
    f8 = ml_dtypes.float8_e4m3
    f16 = np.float16

    x = np.ascontiguousarray(np.asarray(x, dtype=np.float32))
    W1 = np.asarray(W1, dtype=np.float32)
    W2 = np.asarray(W2, dtype=np.float32)
    W0 = np.asarray(W0, dtype=np.float32)

    # [p, k, h] = 32 * W1b[h, k*128+p]
    w1bT = np.ascontiguousarray(
        (W1[:, F:].T * W1B_SCALE).reshape(NF, 128, H).transpose(1, 0, 2).reshape(128, NF * H)
    ).astype(f8)
    # wb [p, j*512+g]: j<8 -> W2[g, j*128+p]; j>=8 -> W0[g, (j-8)*128+p]
    wb = np.empty((128, 12 * 512), dtype=np.float32)
    wb[:, : NH * 512] = W2.T.reshape(NH, 128, F).transpose(1, 0, 2).reshape(128, NH * F)
    wb[:, NH * 512 :] = W0.T.reshape(NF, 128, F).transpose(1, 0, 2).reshape(128, NF * F)
    wbT = np.ascontiguousarray(wb).astype(f16)
    sel = np.zeros((BL, 8 * 512), dtype=np.float32)
    for m in range(8):
        q, bk = divmod(m, 2)
        b0 = 4 * q + 2 * bk
        sel[b0, m * 512 : m * 512 + NI] = 1.0
        sel[b0 + 1, m * 512 + NP : m * 512 + NP + NI] = 1.0
    selT = np.ascontiguousarray(sel).astype(f16)

    in_maps = []
    for i in range(N_CORES):
        xc = x[i * BL : (i + 1) * BL]               # [BL, N, F]
        # wa cols 0:64 = x0 packed [p, f*BL+b]; cols 64: = W1a.T packed
        # [p, k*1024+h]
        wa = np.empty((128, 64 + NF * 1024), dtype=np.float32)
        wa[:, :64] = (
            xc[:, 0, :].T.reshape(NF, 128, BL).transpose(1, 0, 2).reshape(128, NF * BL)
        )
        wa[:, 64:] = (
            W1[:, :F].T.reshape(NF, 128, H).transpose(1, 0, 2).reshape(128, NF * H)
        )
        waT = np.ascontiguousarray(wa).astype(f16)
        pad = np.zeros((BL, NP, F), dtype=np.float32)
        pad[:, :NI, :] = xc[:, 1:, :]
        xiT_full = pad.reshape(BL * NP, F).T        # [512, 4096], col = 256*b + n
        # [p, q, k, c] = xiT_full[k*128+p, q*1024+c]
        xiTc = np.ascontiguousarray(
            xiT_full.reshape(NF, 128, QUADS, 1024)
            .transpose(1, 2, 0, 3)
            .reshape(128, QUADS * NF * 1024)
        ).astype(f8)
        in_maps.append(
            {
                "xiT": xiTc,
                "w1bT": w1bT,
                "waT": waT,
                "wbT": wbT,
                "selT": selT,
            }
        )
    return in_maps


def _gather(results):
    out = np.empty((B, F), dtype=np.float32)
    for i in range(N_CORES):
        out[i * BL : (i + 1) * BL] = results[i]["res"]
    return out


def kernel(x, W1, W2, W0):
    nc = _get_nc()
    in_maps = _make_in_maps(x, W1, W2, W0)
    res = run_bass_kernel_spmd(nc, in_maps, list(range(N_CORES)))
    return _gather(res.results)


def kernel_profiled(x, W1, W2, W0, **trace_kwargs):
    """Like kernel() but with NTFF profiling; returns (out, exec_time_ns)."""
    nc = _get_nc()
    in_maps = _make_in_maps(x, W1, W2, W0)
    res = run_bass_kernel_spmd(
        nc, in_maps, list(range(N_CORES)), trace=True, **trace_kwargs
    )
    return _gather(res.results), res.exec_time_ns
